# revision 1
# baseline (speedup 1.0000x reference)
"""Trainium2 Bass kernel for nn_Cont_InfoNCE (pairwise max cross-correlation + CE loss).

Math: the reference's irfft(F1[i] * conj(F2[j]) / power) is the linear
cross-correlation of the centered rows at every lag, scaled by the positive
constant 1/(power*(T-1)).  max over lags therefore commutes with the scaling,
so dist[i,j] = max_l sum_t f1c[i,t] * f2c[j,t+l] / (1023*s1[i]*s2[j]).

We compute the correlation at all lags as dense fp8e4m3 DoubleRow matmuls on
the tensor engine (fp32 PSUM accumulation; fp8 rounding contributes ~1e-5
relative loss error), max-reduce over lags on the vector engine, and do the
row-wise CE on device.  Sharding: rows of zis across the 8 cores (32 rows
each), zjs replicated; each core emits a partial loss scalar and the host
sums the 8 partials.

Tiling (per core; A = centered local zis rows (32,1024), B = centered zjs):
  Apad[i]    = [0^255, A[i], 0^257]                       (32, 1536) fp8
  Tau[t,i,u] = Apad[i, u+t]          (Hankel gather via DMA from DRAM)
  BT[t,c,j]  = B[j, 128c+t]          (PE transposes, bf16 -> fp8 on copy-out)
  for lam in 0..15, jt in 0..1, ic in 0..7:
    psum[j,ii,d'] += BT[:, 2dc:2dc+2, jtile].T @ Tau[:, ic, u0:u0+256]  (DoubleRow)
      over dc with u0 = 128*(2dc - lam + 9); pair halves are the two
      128-chunks of t, matching the production [P, ksub, free] convention.
  psum[j,ii,d'] equals C[i, j, l] at lag l = 128*lam - 897 - d', covering
  every lag in [-1024, 1023] exactly once (the l = -1024 slot is identically
  0, mirroring the reference's zero-overlap k=1024 slot).
"""

import sys

if "/opt/trn_rl_repo" not in sys.path:
    sys.path.insert(0, "/opt/trn_rl_repo")

from contextlib import ExitStack

import numpy as np

import concourse.bass as bass
import concourse.mybir as mybir
from concourse import bacc, tile
from concourse.bass_utils import run_bass_kernel_spmd
from concourse.masks import make_identity

F32 = mybir.dt.float32
BF16 = mybir.dt.bfloat16
FP8 = mybir.dt.float8e4
I32 = mybir.dt.int32
X = mybir.AxisListType.X
ALU = mybir.AluOpType
ACT = mybir.ActivationFunctionType
DROW = mybir.MatmulPerfMode.DoubleRow

M, T = 256, 1024
NCORES = 8
NLOC = M // NCORES  # 32 rows of zis per core
NIC = 4             # i-rows per i-chunk
NCHUNK = NLOC // NIC  # 8 i-chunks
TAU_U = 1408        # Hankel window extent: covers e0 in [-1, 8], +256 window
APAD = 1536         # 255 zeros + 1024 + 257 zeros


def _rsqrt_scaled(nc, pool, out, ss, k, parts, tag):
    """out = sqrt(k / ss), elementwise on a (parts,1) fp32 column.

    vector.reciprocal (accurate iterative divide) + ACT Sqrt + one Newton
    step to wash out the Sqrt table's loose ULP budget.
    """
    a = pool.tile([parts, 1], F32, tag=tag + "_a")
    nc.vector.reciprocal(a, ss)
    v = pool.tile([parts, 1], F32, tag=tag + "_v")
    nc.vector.tensor_scalar_mul(v, a, float(k))
    y0 = pool.tile([parts, 1], F32, tag=tag + "_y0")
    nc.scalar.sqrt(y0, v)
    ry = pool.tile([parts, 1], F32, tag=tag + "_ry")
    nc.vector.reciprocal(ry, y0)
    t2 = pool.tile([parts, 1], F32, tag=tag + "_t2")
    # t2 = (v * 0.5) * (1/y0)
    nc.vector.scalar_tensor_tensor(t2, in0=v, scalar=0.5, in1=ry, op0=ALU.mult, op1=ALU.mult)
    # out = (y0 * 0.5) + t2
    nc.vector.scalar_tensor_tensor(out, in0=y0, scalar=0.5, in1=t2, op0=ALU.mult, op1=ALU.add)


def _row_stats(nc, pool, in_tile, parts, tag):
    """Returns (negmean, ss) for each row of in_tile, computed on ScalarE.

    ss = sum((x - mean)^2) = sum(x^2) - T*mean^2; the only DVE use is the
    final tiny (parts,1) combine.
    """
    junk1 = pool.tile([parts, T], BF16, tag=tag + "_j1")
    rsum = pool.tile([parts, 1], F32, tag=tag + "_rsum")
    nc.scalar.activation(junk1, in_tile, ACT.Identity, accum_out=rsum)
    junk2 = pool.tile([parts, T], BF16, tag=tag + "_j2")
    ssraw = pool.tile([parts, 1], F32, tag=tag + "_ssraw")
    nc.scalar.activation(junk2, in_tile, ACT.Square, accum_out=ssraw)
    negmean = pool.tile([parts, 1], F32, tag=tag + "_negmean")
    nc.scalar.mul(negmean, rsum, -1.0 / T)
    mu2 = pool.tile([parts, 1], F32, tag=tag + "_mu2")
    nc.scalar.activation(mu2, negmean, ACT.Square)
    ss = pool.tile([parts, 1], F32, tag=tag + "_ss")
    nc.vector.scalar_tensor_tensor(ss, in0=mu2, scalar=-float(T), in1=ssraw, op0=ALU.mult, op1=ALU.add)
    return negmean, ss


def build_nc():
    nc = bacc.Bacc("TRN2", target_bir_lowering=False)
    zis_loc = nc.dram_tensor("zis_loc", [NLOC, T], F32, kind="ExternalInput")
    zjs_full = nc.dram_tensor("zjs_full", [M, T], F32, kind="ExternalInput")
    speeds_loc = nc.dram_tensor("speeds_loc", [NLOC, 1], I32, kind="ExternalInput")
    loss_part = nc.dram_tensor("loss_part", [1, 1], F32, kind="ExternalOutput")

    with tile.TileContext(nc) as tc, ExitStack() as ctx:
        consts = ctx.enter_context(tc.tile_pool(name="consts", bufs=1))
        prep = ctx.enter_context(tc.tile_pool(name="prep", bufs=2))
        dram = ctx.enter_context(tc.tile_pool(name="dram", bufs=1, space="DRAM"))
        taup = ctx.enter_context(tc.tile_pool(name="taup", bufs=3))
        ps_aux = ctx.enter_context(tc.tile_pool(name="ps_aux", bufs=2, space="PSUM"))
        ps_main = ctx.enter_context(tc.tile_pool(name="ps_main", bufs=3, space="PSUM"))

        # ---------------- constants ----------------
        ident_bf = consts.tile([128, 128], BF16)
        make_identity(nc, ident_bf)
        ident_f32 = consts.tile([128, 128], F32)
        make_identity(nc, ident_f32)
        ones_col = consts.tile([NLOC, 1], F32)
        nc.gpsimd.memset(ones_col, 1.0)
        jidx_i = consts.tile([NLOC, M], I32)
        nc.gpsimd.iota(jidx_i, [[1, M]], base=0, channel_multiplier=0)
        jidx_f = consts.tile([NLOC, M], F32)
        nc.scalar.copy(jidx_f, jidx_i)
        sp_i = prep.tile([NLOC, 1], I32)
        nc.sync.dma_start(sp_i, speeds_loc[:, :])
        sp_f = prep.tile([NLOC, 1], F32)
        nc.scalar.copy(sp_f, sp_i)

        # ---------------- A (local zis rows): stats, center -> fp8 Apad ------
        a_in = prep.tile([NLOC, T], F32)
        nc.sync.dma_start(a_in, zis_loc[:, :])
        nega, ss1 = _row_stats(nc, prep, a_in, NLOC, "a")
        r1 = prep.tile([NLOC, 1], F32)
        _rsqrt_scaled(nc, prep, r1, ss1, 1.0 / (T - 1), NLOC, "r1")  # 1/((T-1)*s1)

        apad_sb = prep.tile([NLOC, APAD], FP8)
        nc.gpsimd.memset(apad_sb, 0.0)
        nc.scalar.activation(apad_sb[:, 255:255 + T], a_in, ACT.Identity, bias=nega)
        apad_d = dram.tile([NLOC, APAD], FP8)
        nc.sync.dma_start(apad_d[:, :], apad_sb[:, :])

        # ---------------- B (all zjs rows): stats, center -> bf16 ------------
        bc_tiles = []
        r2_tiles = []
        for jt in range(2):
            b_in = prep.tile([128, T], F32, tag="b_in")
            nc.sync.dma_start(b_in, zjs_full[jt * 128:(jt + 1) * 128, :])
            negb, ss2 = _row_stats(nc, prep, b_in, 128, "b")
            r2 = consts.tile([128, 1], F32, tag=f"r2_{jt}", name=f"r2_{jt}")
            _rsqrt_scaled(nc, prep, r2, ss2, float(T - 1), 128, "r2")  # 1/s2
            r2_tiles.append(r2)
            bc = consts.tile([128, T], BF16, tag=f"bc_{jt}", name=f"bc_{jt}")
            nc.scalar.activation(bc, b_in, ACT.Identity, bias=negb)
            bc_tiles.append(bc)

        # -------- BT[t, c, j] = B[j, 128c+t] via PE transposes, fp8 ----------
        bt8 = consts.tile([128, 8, M], FP8)
        for jt in range(2):
            for c in range(8):
                ps_t = ps_aux.tile([128, 128], BF16, tag="aux")
                nc.tensor.transpose(ps_t, bc_tiles[jt][:, 128 * c:128 * (c + 1)], ident_bf)
                nc.scalar.copy(bt8[:, c, jt * 128:(jt + 1) * 128], ps_t)

        # ---------------- main correlation loop ------------------------------
        cmax_p = [
            consts.tile([128, 16, NLOC], F32, tag=f"cmax_{jt}", name=f"cmax_{jt}")
            for jt in range(2)
        ]
        for ic in range(NCHUNK):
            tau = taup.tile([128, NIC, TAU_U], FP8, tag="tau")
            src = apad_d[NIC * ic:NIC * (ic + 1), 0:TAU_U]
            v = src.unsqueeze(0).broadcast_to((128, NIC, TAU_U))
            lst = v.ap
            lst[0] = [1, 128]  # Hankel: dest partition t reads Apad at +t elements
            v.ap = lst
            nc.sync.dma_start(tau[:, :, :], v)
            for jt in range(2):
                for lp in range(8):  # lambda pairs -> one 2-bank psum tile
                    ps = ps_main.tile([128, 2, NIC, 128], F32, tag="grp")
                    for q in range(2):
                        lam = 2 * lp + q
                        # valid double-chunks: e0 = 2dc - lam + 8 in [-1, 8]
                        dcs = [dc for dc in range(4) if -1 <= 2 * dc - lam + 8 <= 8]
                        for k, dc in enumerate(dcs):
                            u0 = 128 * (2 * dc - lam + 9)
                            rhs = tau[:, :, u0:u0 + 256].rearrange(
                                "p r (i d) -> p i r d", i=2
                            )
                            nc.tensor.matmul(
                                ps[:, q],
                                lhsT=bt8[:, 2 * dc:2 * dc + 2, jt * 128:(jt + 1) * 128],
                                rhs=rhs,
                                perf_mode=DROW,
                                start=(k == 0),
                                stop=(k == len(dcs) - 1),
                            )
                    nc.vector.reduce_max(
                        cmax_p[jt][:, 2 * lp:2 * lp + 2, NIC * ic:NIC * (ic + 1)],
                        ps[:, :, :, :],
                        axis=X,
                    )

        # ---------------- normalize + transpose to (i, j) ---------------------
        dist_t = prep.tile([NLOC, M], F32)
        for jt in range(2):
            cm2 = prep.tile([128, NLOC], F32, tag="cm2")
            nc.vector.reduce_max(cm2, cmax_p[jt].rearrange("p l i -> p i l"), axis=X)
            cms = prep.tile([128, NLOC], F32, tag="cms")
            nc.vector.tensor_scalar(cms, cm2, r2_tiles[jt], None, op0=ALU.mult)
            ps_d = ps_aux.tile([NLOC, 128], F32, tag="aux")
            nc.tensor.transpose(ps_d, cms, ident_f32)
            nc.vector.tensor_scalar(dist_t[:, jt * 128:(jt + 1) * 128], ps_d, r1, None, op0=ALU.mult)

        # ---------------- cross-entropy (sum over local rows) -----------------
        mrow = prep.tile([NLOC, 1], F32)
        nc.vector.reduce_max(mrow, dist_t, axis=X)
        negm = prep.tile([NLOC, 1], F32)
        nc.vector.tensor_scalar_mul(negm, mrow, -1.0)
        expj = prep.tile([NLOC, M], F32)
        sumexp = prep.tile([NLOC, 1], F32)
        nc.scalar.activation(expj, dist_t, ACT.Exp, bias=negm, accum_out=sumexp)
        lse = prep.tile([NLOC, 1], F32)
        nc.scalar.activation(lse, sumexp, ACT.Ln)
        onehot = prep.tile([NLOC, M], F32)
        nc.vector.tensor_scalar(onehot, jidx_f, sp_f, None, op0=ALU.is_equal)
        junk_p = prep.tile([NLOC, M], F32)
        picked = prep.tile([NLOC, 1], F32)
        nc.vector.scalar_tensor_tensor(
            junk_p, in0=dist_t, scalar=1.0, in1=onehot, op0=ALU.mult, op1=ALU.mult, accum_out=picked
        )
        term = prep.tile([NLOC, 1], F32)
        nc.vector.tensor_add(term, lse, mrow)
        term2 = prep.tile([NLOC, 1], F32)
        nc.vector.tensor_sub(term2, term, picked)
        ps_l = ps_aux.tile([1, 1], F32, tag="aux")
        nc.tensor.matmul(ps_l, lhsT=term2, rhs=ones_col, start=True, stop=True)
        lsb = prep.tile([1, 1], F32)
        nc.vector.tensor_copy(lsb, ps_l)
        nc.sync.dma_start(loss_part[:, :], lsb)

    nc.finalize()
    return nc


_NC_CACHE = None
LAST_RESULT = None


def run(zis, zjs, speeds, trace=False):
    global _NC_CACHE, LAST_RESULT
    if _NC_CACHE is None:
        _NC_CACHE = build_nc()
    zis = np.ascontiguousarray(np.asarray(zis), dtype=np.float32)
    zjs = np.ascontiguousarray(np.asarray(zjs), dtype=np.float32)
    sp = np.asarray(speeds).astype(np.int32).reshape(M, 1)
    in_maps = [
        {
            "zis_loc": np.ascontiguousarray(zis[c * NLOC:(c + 1) * NLOC]),
            "zjs_full": zjs,
            "speeds_loc": np.ascontiguousarray(sp[c * NLOC:(c + 1) * NLOC]),
        }
        for c in range(NCORES)
    ]
    res = run_bass_kernel_spmd(_NC_CACHE, in_maps, core_ids=list(range(NCORES)), trace=trace)
    LAST_RESULT = res
    total = sum(float(r["loss_part"][0, 0]) for r in res.results)
    return np.float32(total)


def kernel(zis, zjs, speeds):
    return run(zis, zjs, speeds, trace=False)



# revision 9
# speedup vs baseline: 7.9056x; 7.9056x over previous
"""Trainium2 Bass kernel for nn_Cont_InfoNCE (pairwise max cross-correlation + CE loss).

Math: the reference's irfft(F1[i] * conj(F2[j]) / power) is the linear
cross-correlation of the centered rows at every lag, scaled by the positive
constant 1/(power*(T-1)).  max over lags therefore commutes with the scaling,
so dist[i,j] = max_l sum_t f1c[i,t] * f2c[j,t+l] / (1023*s1[i]*s2[j]).

The host centers the rows, folds 1/s2[j] into B, casts both operands to
fp8e4m3 and pre-builds the transposed B layout BT[t,c,j] = B[j,128c+t]; the
device computes the correlation at all lags as dense fp8 DoubleRow matmuls
on the tensor engine (fp32 PSUM accumulation), max-reduces over lags on the
vector engine, applies the 1/(1023*s1[i]) row scale, and does the row-wise
CE on device.  Sharding: rows of zis across the 8 cores (32 rows each), BT
replicated; each core emits a partial loss scalar and the host sums the 8
partials.

Host->device traffic per call is ~2.4 MB (fp8 operands) instead of the
9.2 MB of raw f32 inputs; the jitted shard_map executor is built once and
cached, so warm calls skip retracing/recompiling entirely.

Tiling (per core; Apad = padded fp8 rows of A):
  Apad[i]    = [0^255, A[i], 0^257]                       (32, 1536) fp8
  Tau[t,i,u] = Apad[i, u+t]          (Hankel gather via DMA from DRAM)
  for lam in 0..15, jt in 0..1, ic in 0..7:
    psum[j,ii,d'] += BT[:, 2dc:2dc+2, jtile].T @ Tau[:, ic, u0:u0+256]  (DoubleRow)
      over dc with u0 = 128*(2dc - lam + 9); pair halves are the two
      128-chunks of t, matching the production [P, ksub, free] convention.
  psum[j,ii,d'] equals C[i, j, l] at lag l = 128*lam - 897 - d', covering
  every lag in [-1024, 1023] exactly once (the l = -1024 slot is identically
  0, mirroring the reference's zero-overlap k=1024 slot).
"""

import sys

if "/opt/trn_rl_repo" not in sys.path:
    sys.path.insert(0, "/opt/trn_rl_repo")

from contextlib import ExitStack

import numpy as np

import concourse.bass as bass
import concourse.mybir as mybir
from concourse import bacc, tile
from concourse.masks import make_identity

F32 = mybir.dt.float32
BF16 = mybir.dt.bfloat16
FP8 = mybir.dt.float8e4
I32 = mybir.dt.int32
NP_FP8 = mybir.dt.np(FP8)
X = mybir.AxisListType.X
ALU = mybir.AluOpType
ACT = mybir.ActivationFunctionType
DROW = mybir.MatmulPerfMode.DoubleRow

M, T = 256, 1024
NCORES = 8
NLOC = M // NCORES  # 32 rows of zis per core
NIC = 4             # i-rows per i-chunk
NCHUNK = NLOC // NIC  # 8 i-chunks
TAU_U = 1408        # Hankel window extent: covers e0 in [-1, 8], +256 window
APAD = 1536         # 255 zeros + 1024 + 257 zeros


USE_COLLECTIVE = True
JBLK = M // NCORES  # 32 j-columns of BT shipped per core when gathering on-device


def build_nc():
    nc = bacc.Bacc("TRN2", target_bir_lowering=False, num_devices=NCORES)
    apad_d = nc.dram_tensor("apad", [NLOC, APAD], FP8, kind="ExternalInput")
    if USE_COLLECTIVE:
        # per-core j-slice of BT; all-gathered on device over the 8 cores
        bts = nc.dram_tensor("bts", [128, 8 * JBLK], FP8, kind="ExternalInput")
    else:
        btd = nc.dram_tensor("btd", [128, 8 * M], FP8, kind="ExternalInput")
    r1_d = nc.dram_tensor("r1", [NLOC, 1], F32, kind="ExternalInput")
    speeds_loc = nc.dram_tensor("speeds_loc", [NLOC, 1], I32, kind="ExternalInput")
    loss_part = nc.dram_tensor("loss_part", [1, 1], F32, kind="ExternalOutput")

    with tile.TileContext(nc) as tc, ExitStack() as ctx:
        consts = ctx.enter_context(tc.tile_pool(name="consts", bufs=1))
        prep = ctx.enter_context(tc.tile_pool(name="prep", bufs=2))
        taup = ctx.enter_context(tc.tile_pool(name="taup", bufs=3))
        ps_aux = ctx.enter_context(tc.tile_pool(name="ps_aux", bufs=2, space="PSUM"))
        ps_main = ctx.enter_context(tc.tile_pool(name="ps_main", bufs=3, space="PSUM"))

        # ---------------- constants / inputs ----------------
        ident_f32 = consts.tile([128, 128], F32)
        make_identity(nc, ident_f32)
        ones_col = consts.tile([NLOC, 1], F32)
        nc.gpsimd.memset(ones_col, 1.0)
        jidx_i = consts.tile([NLOC, M], I32)
        nc.gpsimd.iota(jidx_i, [[1, M]], base=0, channel_multiplier=0)
        jidx_f = consts.tile([NLOC, M], F32)
        nc.scalar.copy(jidx_f, jidx_i)
        sp_i = prep.tile([NLOC, 1], I32)
        nc.sync.dma_start(sp_i, speeds_loc[:, :])
        sp_f = prep.tile([NLOC, 1], F32)
        nc.scalar.copy(sp_f, sp_i)
        r1 = prep.tile([NLOC, 1], F32)
        nc.sync.dma_start(r1, r1_d[:, :])
        bt8 = consts.tile([128, 8, M], FP8)
        if USE_COLLECTIVE:
            # bounce the ExternalInput slice into a DRAM pool tile
            # (collectives may not touch kernel I/O tensors), all-gather the
            # 8 j-slices, then repack the canonical [t, c, j] SBUF layout.
            # TileContext tracks the bounce tiles and orders
            # dma -> collective -> repack automatically.
            dram = ctx.enter_context(tc.tile_pool(name="dram", bufs=1, space="DRAM"))
            bt_bounce = dram.tile([128, 8 * JBLK], FP8)
            bt_gather = dram.tile([NCORES * 128, 8 * JBLK], FP8)
            nc.gpsimd.dma_start(bt_bounce[:, :], bts[:, :])
            nc.gpsimd.collective_compute(
                "AllGather",
                mybir.AluOpType.bypass,
                replica_groups=[list(range(NCORES))],
                ins=[bt_bounce.opt()],
                outs=[bt_gather.opt()],
            )
            for r in range(NCORES):
                nc.sync.dma_start(
                    bt8[:, :, JBLK * r:JBLK * (r + 1)],
                    bt_gather[128 * r:128 * (r + 1), :].rearrange(
                        "p (c j) -> p c j", c=8
                    ),
                )
        else:
            nc.sync.dma_start(bt8[:, :, :], btd[:, :].rearrange("p (c j) -> p c j", c=8))

        # ---------------- main correlation loop ------------------------------
        cmax_p = [
            consts.tile([128, 16, NLOC], F32, tag=f"cmax_{jt}", name=f"cmax_{jt}")
            for jt in range(2)
        ]
        for ic in range(NCHUNK):
            tau = taup.tile([128, NIC, TAU_U], FP8, tag="tau")
            src = apad_d[NIC * ic:NIC * (ic + 1), 0:TAU_U]
            v = src.unsqueeze(0).broadcast_to((128, NIC, TAU_U))
            lst = v.ap
            lst[0] = [1, 128]  # Hankel: dest partition t reads Apad at +t elements
            v.ap = lst
            nc.sync.dma_start(tau[:, :, :], v)
            for jt in range(2):
                for lp in range(8):  # lambda pairs -> one 2-bank psum tile
                    ps = ps_main.tile([128, 2, NIC, 128], F32, tag="grp")
                    for q in range(2):
                        lam = 2 * lp + q
                        # valid double-chunks: e0 = 2dc - lam + 8 in [-1, 8]
                        dcs = [dc for dc in range(4) if -1 <= 2 * dc - lam + 8 <= 8]
                        for k, dc in enumerate(dcs):
                            u0 = 128 * (2 * dc - lam + 9)
                            rhs = tau[:, :, u0:u0 + 256].rearrange(
                                "p r (i d) -> p i r d", i=2
                            )
                            nc.tensor.matmul(
                                ps[:, q],
                                lhsT=bt8[:, 2 * dc:2 * dc + 2, jt * 128:(jt + 1) * 128],
                                rhs=rhs,
                                perf_mode=DROW,
                                start=(k == 0),
                                stop=(k == len(dcs) - 1),
                            )
                    nc.vector.reduce_max(
                        cmax_p[jt][:, 2 * lp:2 * lp + 2, NIC * ic:NIC * (ic + 1)],
                        ps[:, :, :, :],
                        axis=X,
                    )

        # ---------------- reduce over lag groups + transpose to (i, j) --------
        dist_t = prep.tile([NLOC, M], F32)
        for jt in range(2):
            cm2 = prep.tile([128, NLOC], F32, tag="cm2")
            nc.vector.reduce_max(cm2, cmax_p[jt].rearrange("p l i -> p i l"), axis=X)
            ps_d = ps_aux.tile([NLOC, 128], F32, tag="aux")
            nc.tensor.transpose(ps_d, cm2, ident_f32)
            nc.vector.tensor_scalar(dist_t[:, jt * 128:(jt + 1) * 128], ps_d, r1, None, op0=ALU.mult)

        # ---------------- cross-entropy (sum over local rows) -----------------
        mrow = prep.tile([NLOC, 1], F32)
        nc.vector.reduce_max(mrow, dist_t, axis=X)
        negm = prep.tile([NLOC, 1], F32)
        nc.vector.tensor_scalar_mul(negm, mrow, -1.0)
        expj = prep.tile([NLOC, M], F32)
        sumexp = prep.tile([NLOC, 1], F32)
        nc.scalar.activation(expj, dist_t, ACT.Exp, bias=negm, accum_out=sumexp)
        lse = prep.tile([NLOC, 1], F32)
        nc.scalar.activation(lse, sumexp, ACT.Ln)
        onehot = prep.tile([NLOC, M], F32)
        nc.vector.tensor_scalar(onehot, jidx_f, sp_f, None, op0=ALU.is_equal)
        junk_p = prep.tile([NLOC, M], F32)
        picked = prep.tile([NLOC, 1], F32)
        nc.vector.scalar_tensor_tensor(
            junk_p, in0=dist_t, scalar=1.0, in1=onehot, op0=ALU.mult, op1=ALU.mult, accum_out=picked
        )
        term = prep.tile([NLOC, 1], F32)
        nc.vector.tensor_add(term, lse, mrow)
        term2 = prep.tile([NLOC, 1], F32)
        nc.vector.tensor_sub(term2, term, picked)
        ps_l = ps_aux.tile([1, 1], F32, tag="aux")
        nc.tensor.matmul(ps_l, lhsT=term2, rhs=ones_col, start=True, stop=True)
        lsb = prep.tile([1, 1], F32)
        nc.vector.tensor_copy(lsb, ps_l)
        nc.sync.dma_start(loss_part[:, :], lsb)

    nc.finalize()
    return nc


_RUNNER = None
LAST_RESULT = None


def _make_runner():
    """Build the Bass module and a persistently-cached jitted executor.

    run_bass_kernel_spmd rebuilds its jit closure on every call, so each
    call re-traces, re-runs BIR verify/optimise and XLA compile (~0.5 s)
    and re-fetches the sharded output once per core.  Here the
    jax.jit(shard_map(...)) wrapper is built exactly once; warm calls hit
    the pjit C++ fast path and do a single host<->device round trip.
    """
    import jax
    from jax.experimental.shard_map import shard_map
    from jax.sharding import Mesh, PartitionSpec

    from concourse import bass2jax

    nc = build_nc()
    bass2jax.install_neuronx_cc_hook()
    assert nc.dbg_addr is None or not nc.dbg_callbacks
    partition_name = nc.partition_id_tensor.name if nc.partition_id_tensor else None

    in_names, out_names, out_avals = [], [], []
    for alloc in nc.m.functions[0].allocations:
        if not isinstance(alloc, mybir.MemoryLocationSet):
            continue
        name = alloc.memorylocations[0].name
        if alloc.kind == "ExternalInput":
            if name != partition_name:
                in_names.append(name)
        elif alloc.kind == "ExternalOutput":
            out_names.append(name)
            out_avals.append(
                jax.core.ShapedArray(tuple(alloc.tensor_shape), mybir.dt.np(alloc.dtype))
            )
    n_params = len(in_names)
    n_outs = len(out_avals)
    all_in_names = tuple(in_names + out_names + ([partition_name] if partition_name else []))
    donate = tuple(range(n_params, n_params + n_outs))

    def _body(*args):
        operands = list(args)
        if partition_name is not None:
            operands.append(bass2jax.partition_id_tensor())
        outs = bass2jax._bass_exec_p.bind(
            *operands,
            out_avals=tuple(out_avals),
            in_names=all_in_names,
            out_names=tuple(out_names),
            lowering_input_output_aliases=(),
            sim_require_finite=True,
            sim_require_nnan=True,
            nc=nc,
        )
        return tuple(outs)

    devices = jax.devices()[:NCORES]
    mesh = Mesh(np.asarray(devices), ("core",))
    in_specs = (PartitionSpec("core"),) * (n_params + n_outs)
    out_specs = (PartitionSpec("core"),) * n_outs
    sharded = jax.jit(
        shard_map(_body, mesh=mesh, in_specs=in_specs, out_specs=out_specs, check_rep=False),
        donate_argnums=donate,
        keep_unused=True,
    )
    zero_shapes = [
        ((NCORES * a.shape[0],) + tuple(a.shape[1:]), a.dtype) for a in out_avals
    ]

    def call(concat_inputs):
        """concat_inputs: dict name -> global (NCORES*rows, ...) array."""
        ins = [concat_inputs[name] for name in in_names]
        zeros = [np.zeros(s, d) for s, d in zero_shapes]
        out_arrs = sharded(*ins, *zeros)
        return [np.asarray(o) for o in out_arrs]

    return call


_F16_TO_FP8 = None


def _to_fp8(x):
    """Fast f32 -> fp8e4m3 via f16 + 64K lookup (ml_dtypes scalar cast is slow)."""
    global _F16_TO_FP8
    if _F16_TO_FP8 is None:
        all16 = np.arange(65536, dtype=np.uint16).view(np.float16)
        _F16_TO_FP8 = all16.astype(np.float32).astype(NP_FP8).view(np.uint8)
    idx = x.astype(np.float16).view(np.uint16)
    return _F16_TO_FP8[idx].view(NP_FP8)


def _center(z):
    zc = z - z.mean(axis=-1, keepdims=True, dtype=np.float64).astype(np.float32)
    ss = np.einsum("ij,ij->i", zc, zc)
    s = np.sqrt(ss / (T - 1))
    return zc, np.where(s == 0.0, 1.0, s)


def _host_prep(zis, zjs, speeds):
    """Center rows, fold the per-row scales, cast to fp8, build BT layout."""
    f1c, s1 = _center(np.asarray(zis, dtype=np.float32))
    r1 = (1.0 / ((T - 1) * s1)).astype(np.float32).reshape(M, 1)

    f2c, s2 = _center(np.asarray(zjs, dtype=np.float32))
    b = _to_fp8(f2c * (1.0 / s2)[:, None])  # (M, T), 1/s2 folded in

    apad = np.zeros((M, APAD), NP_FP8)
    apad[:, 255:255 + T] = _to_fp8(f1c)
    # BT[t, c, j] = B[j, 128c + t]
    bt = np.ascontiguousarray(b.reshape(M, 8, 128).transpose(2, 1, 0)).reshape(128, 8 * M)
    sp = np.ascontiguousarray(np.asarray(speeds).astype(np.int32).reshape(M, 1))
    return apad, bt, r1, sp


def run(zis, zjs, speeds, trace=False):
    global _RUNNER
    if _RUNNER is None:
        _RUNNER = _make_runner()
    apad, bt, r1, sp = _host_prep(zis, zjs, speeds)
    concat_inputs = {
        "apad": apad,                      # row-block sharding == identity
        "r1": r1,
        "speeds_loc": sp,
    }
    if USE_COLLECTIVE:
        # core c ships BT[:, :, JBLK*c:JBLK*(c+1)] as its shard
        bt4 = bt.reshape(128, 8, NCORES, JBLK)
        concat_inputs["bts"] = np.ascontiguousarray(
            bt4.transpose(2, 0, 1, 3)
        ).reshape(NCORES * 128, 8 * JBLK)
    else:
        concat_inputs["btd"] = np.tile(bt, (NCORES, 1))
    outs = _RUNNER(concat_inputs)
    loss_parts = outs[0].reshape(NCORES)
    return np.float32(float(loss_parts.sum()))


def kernel(zis, zjs, speeds):
    return run(zis, zjs, speeds, trace=False)
